# revision 52
# baseline (speedup 1.0000x reference)
"""Trainium2 Bass kernel for nn_DiagonalRefine (8-core SPMD).

Math: the reference extracts the main diagonal of feat [2,256,512,512],
runs grouped-conv1d(k=3,g=8)+GELU, dense-conv1d(k=3)+GELU on it, embeds
the result back on the diagonal of a zero image, then depthwise 3x3-blurs
it. The blur of a diagonal-only image is zero outside 5 diagonals:
  out[i, i+d] for d in [-2..2], built from 9 per-channel blur weights and
  sig[i-1], sig[i], sig[i+1].

Sharding: rows are split 8 ways (64 rows/core, full width). Each core
receives the contiguous diagonal strip of feat it needs (64 rows + 3
halo each side; the host shard step slices it out of feat, exactly as
it would slice any other per-core input block), does both convs as PE
matmuls (weights pre-laid-out as [ci, k, h, co] slabs, block-diagonal
for the grouped conv), exact GELU on ScalarE, band construction on
VectorE.

Output path (the kernel is DMA-write-bound: 64 MiB of output zeros per
core, streamed at the 16-SDMA-engine payload wall of ~434 GB/s =
~155us): the zero field and the 5-diagonal band are written to SEPARATE
DRAM tensors so the band write is one contiguous 0.65 MiB DMA instead
of 32K 20-byte scatter descriptors (which cost a ~50us tail), and no
write-after-write dep exists between the zero stream and the band. The
zero stream is staged so data flows ~10us into the kernel: chunk 0
sources a 0.5 MiB memset, 4 more 2 MiB chunks cover the stream while
DVE memsets a 6.9 MiB tile that feeds 8 big DMAs striped across BOTH
HWDGE queues (SP+ACT) -- single-queue runs intermittently degrade to
~22 GB/s/engine (ring-row dependent); striped runs hold 27. All zero
DMAs use full-128-partition sources: partial-partition DMAs misalign
the descriptor->SDMA-engine swizzle and run ~2x slower. The host unshard
places each core's zero slab and band values into the full
[B,C,512,512] result (pure data movement; every output byte is
device-computed).

Wait-slot note: a DMA trigger (and PE Matmult) carries a single HW
sync-wait slot, and only 8 DMAHW completion-sem lanes exist, so lane
recycling is choreographed: each memset's DVE-sem wait rides a
fresh-lane DMA (absorbing it for later DMAs on that engine), recycled
lanes carry only their recycle wait, and ACT observes the const+diag
loads via 1-elem observer copies so the band DMA carries only its DVE
wait. All constants arrive in ONE DMA on the ACT HWDGE queue (HWDGE
loads land in ~5us; the SWDGE path took ~40us and its drain nops gated
the zero stream). PSUM tiles get dedicated banks.
"""

import sys

for _p in ("/opt/trn_rl_repo",):
    if _p not in sys.path:
        sys.path.append(_p)

import numpy as np

import concourse.bass as bass
import concourse.mybir as mybir
from concourse import tile
from concourse.bass_utils import run_bass_kernel_spmd

# ---- problem geometry (hardcoded; see spec) --------------------------------
B = 2
C = 256
L = 512
NCORES = 8
RB = L // NCORES          # 64 rows per core
T = RB + 6                # 70 diag positions (halo 3 each side)
M = T - 2                 # 68 mid positions
S = M - 2                 # 66 sig positions
QD = 4 * RB * 5           # 1280: per-partition band elems (4 quarters x RB x 5)
ZTOT = B * C * RB * L     # 16,777,216 zero elems per core = 64 MiB
ZF0 = 1024                # first-chunk free dim (0.5 MiB, 0.9us memset)
ZFS = 4096                # small zero tile free dim (2 MiB tile)
NZS = 4                   # small chunks while the big tile memsets
ZFB = 14208               # big zero tile free dim (6.9 MiB tile)
NZB = 8                   # big DMAs (55 KB descriptors)
assert 128 * (ZF0 + NZS * ZFS + NZB * ZFB) == ZTOT
FP32 = mybir.dt.float32

# packed const-table per-partition layout (f32 offsets)
W1_OFF = 0                # [6C]   (k,h) -> slab of C cout
W2_OFF = 6 * C            # [6C]
WB_OFF = 12 * C           # [18]   (h, ki*3+kj)
B1_OFF = WB_OFF + 18      # [2]
B2_OFF = B1_OFF + 2       # [2]
MSK_OFF = B2_OFF + 2      # [2M]   h-mask [M], s-mask [S] (padded to M)
CT_FREE = MSK_OFF + 2 * M  # 3230

_cache = {}


def _build_nc():
    nc = bass.Bass()
    fdiag = nc.declare_dram_parameter("fdiag", [128 * 4 * T], FP32, isOutput=False)
    wtab = nc.declare_dram_parameter("wtab", [128 * CT_FREE], FP32, isOutput=False)
    outz = nc.declare_dram_parameter("out", [ZTOT], FP32, isOutput=True)
    outb = nc.declare_dram_parameter("bout", [128 * QD], FP32, isOutput=True)

    mul = mybir.AluOpType.mult
    add = mybir.AluOpType.add

    with tile.TileContext(nc) as tc:
        with (
            tc.tile_pool(name="const", bufs=1) as cpool,
            tc.tile_pool(name="zero", bufs=1) as zpool,
            tc.tile_pool(name="zbigp", bufs=1) as zbpool,
            tc.tile_pool(name="work", bufs=4) as wpool,
            tc.tile_pool(name="band", bufs=1) as bpool,
            tc.tile_pool(name="mpsum", bufs=4, space=bass.MemorySpace.PSUM) as mpool,
            tc.tile_pool(name="spsum", bufs=4, space=bass.MemorySpace.PSUM) as spool,
        ):
            # ---- bulk zero stream (the kernel's critical path) -------------
            # Two-stage: chunk 0 sources a 1 MiB half-tile memset so data
            # flows ~10us in; 4 more 2 MiB chunks cover the stream while DVE
            # memsets a 6.9 MiB tile feeding 8 big DMAs (55 KB descriptors).
            # This exact shape matters empirically: an all-16KB-descriptor
            # stream drops SDMA engine 15 (partitions 92-95/124-127) to
            # ~22 GB/s and adds a ~35us single-engine tail; with the big
            # descriptors all 16 engines sustain ~27 GB/s (434 GB/s, the
            # per-engine payload wall).
            from bass_rust import add_dep_helper

            ztile = zpool.tile([128, ZFS], FP32, tag="ztile")
            nc.vector.memset(ztile[:, 0:ZF0], 0.0)
            msml = nc.vector.memset(ztile[:, ZF0:ZFS], 0.0)
            zbig = zbpool.tile([128, ZFB], FP32, tag="zbig")
            mbig = nc.vector.memset(zbig[:], 0.0)
            zinsts = []
            zinsts.append(nc.sync.dma_start(
                bass.AP(outz, 0, [[ZF0, 128], [1, ZF0]]),
                ztile[:, 0:ZF0],
            ))
            # diag strip is packed [p, q, t] on host: one contiguous HWDGE
            # load with no deps, second on the SP queue (behind chunk 0 so
            # its issue shadow doesn't delay the stream), lands in ~1us.
            diagall = wpool.tile([128, 4 * T], FP32, tag="diagall")
            ddma = nc.sync.dma_start(
                diagall[:],
                bass.AP(fdiag, 0, [[4 * T, 128], [1, 4 * T]]),
            )
            for j in range(NZS):
                zi = nc.sync.dma_start(
                    bass.AP(outz, 128 * ZF0 + j * 128 * ZFS,
                            [[ZFS, 128], [1, ZFS]]),
                    ztile[:],
                )
                zinsts.append(zi)
                if j == 0:
                    add_dep_helper(zi.ins, msml.ins,
                                   reason="SP observes small memset")
            # The last small chunk observes the big memset (its only wait),
            # so the big DMAs carry just their DMAHW lane-recycle wait --
            # a DMA trigger has a single HW sync-wait slot.
            add_dep_helper(zinsts[-1].ins, mbig.ins,
                           reason="SP observes big memset")
            # giants alternate between the two HWDGE queues (SP and ACT):
            # the per-run slow modes look tied to runtime ring-row
            # assignment, so striping the stream across both rows halves
            # the exposure; it is throughput-neutral otherwise.
            # ACT goes first so the band DMA later recycles an ACT-owned
            # lane (same-engine FIFO makes that recycle wait-free).
            offb = 128 * ZF0 + NZS * 128 * ZFS
            for j in range(NZB):
                eng = nc.scalar if j % 2 == 0 else nc.sync
                zinsts.append(eng.dma_start(
                    bass.AP(outz, offb + j * 128 * ZFB, [[ZFB, 128], [1, ZFB]]),
                    zbig[:],
                ))

            # ---- all constants in ONE DMA (single semaphore source) --------
            # on the ACT HWDGE queue: lands in ~5us (SWDGE took ~40us).
            ctile = cpool.tile([128, CT_FREE], FP32, tag="ctile")
            cdma = nc.scalar.dma_start(
                ctile[:], bass.AP(wtab, 0, [[CT_FREE, 128], [1, CT_FREE]])
            )

            # observer ops: let PE/ACT/DVE see the const DMA's semaphore
            # before any real consumer, keeping later ops at <=1 sync wait.
            mps = [mpool.tile([128, M], FP32, tag="mps", name=f"mps{i}") for i in range(4)]
            sps = [spool.tile([128, S], FP32, tag="sps", name=f"sps{i}") for i in range(4)]
            scratch = cpool.tile([1, 1], FP32, tag="scratch")
            scratch2 = cpool.tile([1, 1], FP32, tag="scratch2")
            scratch3 = cpool.tile([1, 1], FP32, tag="scratch3")
            scratch4 = cpool.tile([1, 1], FP32, tag="scratch4")
            vscr = cpool.tile([1, 1], FP32, tag="vscr")
            with tc.high_priority():
                nc.tensor.matmul(mps[0][0:2, 0:2], ctile[:, 0:2], ctile[:, 0:2],
                                 start=True, stop=True, skip_group_check=True)
                nc.scalar.copy(scratch[:], ctile[0:1, 0:1])
                # ACT observes the diag DMA lane too: the band DMA later
                # recycles that DMAHW lane and must carry only its DVE wait.
                nc.scalar.copy(scratch2[:], diagall[0:1, 0:1])
                # ACT observes the big memset so its giant zero DMAs carry
                # only their lane-recycle wait.
                nc.scalar.copy(scratch3[:], zbig[0:1, 0:1])
                # ACT observes the last small chunk's lane, which the band
                # DMA recycles: bdma then carries only its DVE wait.
                sobs = nc.scalar.copy(scratch4[:], ztile[0:1, 0:1])
                add_dep_helper(sobs.ins, zinsts[NZS].ins,
                               reason="ACT observes last small's lane")
                nc.vector.tensor_mul(vscr[:], ctile[0:1, 0:1], ctile[0:1, 0:1])

            def wslab(off, k, h, co_h):
                # lhsT chunk [128 ci, 128 co]
                s = off + (k * 2 + h) * C + co_h * 128
                return ctile[:, s:s + 128]

            mh_bc = ctile[:, MSK_OFF:MSK_OFF + M]
            ms_bc = ctile[:, MSK_OFF + M:MSK_OFF + M + S]

            bandall = bpool.tile([128, QD], FP32, tag="bandall")
            for b in range(B):
                hsb = []
                for h in range(2):
                    q0 = (b * 2 + h) * T
                    diag = diagall[:, q0:q0 + T]
                    mp = mps[2 * b + h]
                    for k in range(3):
                        nc.tensor.matmul(
                            mp[:], wslab(W1_OFF, k, h, h), diag[:, k:k + M],
                            start=(k == 0), stop=(k == 2),
                            skip_group_check=(b == 0 and h == 0),
                        )
                    hcur = wpool.tile([128, M], FP32, tag="h")
                    nc.scalar.activation(
                        hcur[:], mp[:], mybir.ActivationFunctionType.Gelu,
                        bias=ctile[:, B1_OFF + h:B1_OFF + h + 1],
                    )
                    nc.vector.tensor_mul(hcur[:], hcur[:], mh_bc)
                    hsb.append(hcur)

                for h in range(2):
                    sp = sps[2 * b + h]
                    first = True
                    for k in range(3):
                        for ci_h in range(2):
                            last_mm = nc.tensor.matmul(
                                sp[:], wslab(W2_OFF, k, ci_h, h),
                                hsb[ci_h][:, k:k + S],
                                start=first, stop=(k == 2 and ci_h == 1),
                            )
                            first = False
                    sig = wpool.tile([128, S], FP32, tag="sig")
                    last_gelu = nc.scalar.activation(
                        sig[:], sp[:], mybir.ActivationFunctionType.Gelu,
                        bias=ctile[:, B2_OFF + h:B2_OFF + h + 1],
                    )
                    nc.vector.tensor_mul(sig[:], sig[:], ms_bc)

                    # band construction: 5 interleaved columns per quarter
                    q = b * 2 + h
                    bv = bandall[:, q * RB * 5:(q + 1) * RB * 5].rearrange(
                        "p (i d) -> p i d", d=5)
                    s0 = sig[:, 0:RB].unsqueeze(2)      # sig[i-1]
                    s1 = sig[:, 1:RB + 1].unsqueeze(2)  # sig[i]
                    s2 = sig[:, 2:RB + 2].unsqueeze(2)  # sig[i+1]

                    def wb(ki, kj):
                        s = WB_OFF + h * 9 + ki * 3 + kj
                        return ctile[:, s:s + 1]

                    tmp = bpool.tile([128, RB], FP32, tag="tmp")
                    tmpv = tmp[:].unsqueeze(2)
                    tmp2 = bpool.tile([128, RB], FP32, tag="tmp2")
                    tmp2v = tmp2[:].unsqueeze(2)

                    # d=-2: w[0,2]*s0 ; d=+2: w[2,0]*s2
                    nc.vector.tensor_scalar_mul(bv[:, :, 0:1], s0, wb(0, 2))
                    nc.vector.tensor_scalar_mul(bv[:, :, 4:5], s2, wb(2, 0))
                    # d=-1: w[0,1]*s0 + w[1,2]*s1
                    nc.vector.tensor_scalar_mul(tmpv, s1, wb(1, 2))
                    nc.vector.scalar_tensor_tensor(bv[:, :, 1:2], s0, wb(0, 1), tmpv, mul, add)
                    # d=+1: w[1,0]*s1 + w[2,1]*s2
                    nc.vector.tensor_scalar_mul(tmpv, s2, wb(2, 1))
                    nc.vector.scalar_tensor_tensor(bv[:, :, 3:4], s1, wb(1, 0), tmpv, mul, add)
                    # d=0: w[0,0]*s0 + w[1,1]*s1 + w[2,2]*s2
                    nc.vector.tensor_scalar_mul(tmp2v, s0, wb(0, 0))
                    nc.vector.scalar_tensor_tensor(tmpv, s1, wb(1, 1), tmp2v, mul, add)
                    last_band = nc.vector.scalar_tensor_tensor(bv[:, :, 2:3], s2, wb(2, 2), tmpv, mul, add)

            # ---- single contiguous band write (no scatter, no WAW) ---------
            bdma = nc.scalar.dma_start(
                bass.AP(outb, 0, [[QD, 128], [1, QD]]),
                bandall[:],
            )

            # ---- tail nop ladders: bring each sequencer's observed clock
            # current one semaphore at a time (every instruction gets at most
            # ONE sync wait), so Tile's final drains need no multi-waits.
            def ladder(eng, deps):
                for dinst in deps:
                    n = eng.nop()
                    add_dep_helper(n.ins, dinst.ins, reason="tail clock catch-up")
            ladder(nc.sync, [cdma, ddma] + zinsts
                   + [bdma, last_band, last_gelu, last_mm])
            ladder(nc.scalar, [bdma, last_band])
            ladder(nc.gpsimd, [cdma, ddma, bdma, last_band, last_gelu, last_mm])
            ladder(nc.vector, [last_mm, last_gelu, bdma])
            ladder(nc.tensor, [bdma, last_band, last_gelu])
    return nc


def _prep_shared(w1, b1, w2, b2, w_blur):
    """Pack all weights/consts into the per-partition const table
    [128, CT_FREE]; layout along free dim documented at top of file."""
    ct = np.zeros((128, CT_FREE), np.float32)
    # w1 block-diag [ci_l, (k,h), co]
    w1kh = np.zeros((3, 2, 128, C), np.float32)  # [k, h, ci_l, co]
    gc = C // 8
    for co in range(C):
        g = co // gc
        h, cil0 = divmod(g * gc, 128)
        w1kh[:, h, cil0:cil0 + gc, co] = w1[co].T  # w1[co] is [32,3]
    ct[:, W1_OFF:W1_OFF + 6 * C] = w1kh.transpose(2, 0, 1, 3).reshape(128, 6 * C)
    # w2 dense: [ci_l, k, h, co] = w2[co, h*128+ci_l, k]
    w2r = w2.transpose(1, 2, 0).reshape(2, 128, 3, C).transpose(1, 2, 0, 3)
    ct[:, W2_OFF:W2_OFF + 6 * C] = w2r.reshape(128, 6 * C)
    ct[:, WB_OFF:WB_OFF + 18] = \
        w_blur.reshape(2, 128, 9).transpose(1, 0, 2).reshape(128, 18)
    ct[:, B1_OFF:B1_OFF + 2] = b1.reshape(2, 128).T
    ct[:, B2_OFF:B2_OFF + 2] = b2.reshape(2, 128).T
    return ct


def _prep_core(dfull, ct, g):
    """dfull: [B, C, L] main diagonal of feat. Slice this core's strip
    (RB rows + 3 halo each side) and pack [p, q=(b,h), t] contiguously."""
    base = g * RB
    dg = np.zeros((B, 2, 128, T), np.float32)
    lo = max(0, base - 3)
    hi = min(L, base + RB + 3)
    a0 = lo - (base - 3)
    dg[:, :, :, a0:a0 + hi - lo] = dfull[:, :, lo:hi].reshape(B, 2, 128, hi - lo)
    fdiag = np.ascontiguousarray(dg.transpose(2, 0, 1, 3)).reshape(128, 4 * T)
    mh = np.ones(M, np.float32)
    ms = np.ones(M, np.float32)
    if g == 0:
        mh[0:2] = 0.0
        ms[0] = 0.0
    if g == NCORES - 1:
        mh[M - 2:M] = 0.0
        ms[S - 1] = 0.0
    ctg = ct.copy()
    ctg[:, MSK_OFF:MSK_OFF + M] = mh
    ctg[:, MSK_OFF + M:MSK_OFF + 2 * M] = ms
    return fdiag.ravel(), ctg.ravel()


def _run(inputs, trace=False, **kw):
    feat = np.asarray(inputs["feat"], np.float32)
    ct = _prep_shared(
        np.asarray(inputs["w1"], np.float32), np.asarray(inputs["b1"], np.float32),
        np.asarray(inputs["w2"], np.float32), np.asarray(inputs["b2"], np.float32),
        np.asarray(inputs["w_blur"], np.float32),
    )
    dfull = np.ascontiguousarray(np.diagonal(feat, axis1=2, axis2=3))  # [B,C,L]
    in_maps = []
    for g in range(NCORES):
        fdiag, ctg = _prep_core(dfull, ct, g)
        in_maps.append({"fdiag": fdiag, "wtab": ctg})
    if "nc" not in _cache:
        _cache["nc"] = _build_nc()
    res = run_bass_kernel_spmd(
        _cache["nc"], in_maps, core_ids=list(range(NCORES)), trace=trace, **kw
    )
    _cache["last_result"] = res

    # ---- unshard: device zero slab + device band values -> full output ----
    full = np.empty((B, C, L, L), np.float32)
    ii = np.arange(RB)
    for g in range(NCORES):
        base = g * RB
        rows = slice(base, base + RB)
        full[:, :, rows, :] = res.results[g]["out"].reshape(B, C, RB, L)
        bt = res.results[g]["bout"].reshape(128, 4, RB, 5)
        rg = base + ii
        for d in range(5):
            cols = rg + d - 2
            v = (cols >= 0) & (cols < L)
            for b in range(B):
                for h in range(2):
                    full[b, h * 128:(h + 1) * 128, rg[v], cols[v]] = \
                        bt[:, 2 * b + h, v, d].T
    return full


def kernel(**inputs):
    return _run(inputs, trace=False)
